# revision 64
# baseline (speedup 1.0000x reference)
"""Trainium2 Bass kernel: GQA multi-head attention block (nn_MultiHeadAttention).

Full-input contract: kernel(**inputs) takes the unsharded inputs and returns the
full [B, T, D] output. Internally shards across 8 NeuronCores as
2 (batch / data axis) x 4 (head groups / model axis): each core processes one
batch element and 12 q heads (2 kv heads) including the row-shard of the output
projection; the host sums the 4 model-parallel partial outputs per batch.

Per-core compute layout ("transposed attention"):
  - host passes x^T [D, T] so projections emit Q^T/K^T [d, t] directly
    (features on partitions).
  - S^T tile [tk=128, tq<=512] = single matmul (contraction d=128).
  - soft logit cap: 30*tanh(logits/30); softmax uses the fixed max 30
    (tanh bounds logits to [-30,30], so no row-max pass is needed).
  - causal: upper-triangular tiles are skipped structurally; diagonal-band
    tiles are column-trimmed to the causal support (kept >=256 wide); the
    -1e9 mask is ACCUMULATED INTO THE LOGIT PSUM by a second matmul
    (identity x trimask) so no vector-engine op sits in the softmax chain
    (tanh saturates to -1, exp maps it to ~0 -- exact).
  - rope: rotate-half is a PE matmul against a [128,128] permutation matrix;
    the sign lives in the sin table; the elementwise combine runs on DVE in
    bf16 (2-byte fast path).
  - softmax denominator via ones-column matmul (PE), 1/r broadcast across
    partitions on the Pool engine.
  - every matmul operand is bf16 (streams, weights, Q/K/V/P/ot, constants);
    PSUM accumulation and the softmax chain stay f32.

Schedule (PE is in-order, so filler work is EMITTED inside the tanh+exp wait
window of each attention m-step):
  - block 0 projects Q before K/V (it is DMA-starved; Q's operands arrive
    first); blocks 1..nKB-1 run K/V then Q for all heads.
  - last block (context A): K/V + K-rope + Q of heads 0..5 + V-transposes.
  - context B: per head h: attention; Q+rope of heads 6..11 and (from head 5)
    O-projection ti-units are generators, drained one unit per wait window.
  - output stores ride the Act engine's DGE queue; SP carries only loads.
"""

import sys
from contextlib import ExitStack
from dataclasses import dataclass

for _p in (
    "/opt/trn_rl_repo",
    "/opt/pypackages",
    "/root/.axon_site/_ro/trn_rl_repo",
    "/root/.axon_site/_ro/pypackages",
):
    if _p not in sys.path:
        sys.path.insert(0, _p)

import numpy as np  # noqa: E402

import concourse.mybir as mybir  # noqa: E402
import concourse.tile as tile  # noqa: E402
from concourse import bacc, bass_utils  # noqa: E402

MULT = 0.08838834764831845  # 1/sqrt(128)
MAXA = 30.0  # tanh logit cap
NEG = -1.0e9  # additive logit mask
ROPE_BASE = 10000.0
HD = 128  # head dim (fixed: rope halves assume 64/64)

F32 = mybir.dt.float32
AF = mybir.ActivationFunctionType


@dataclass(frozen=True)
class Cfg:
    T: int = 1024  # tokens per core
    D: int = 6144  # model dim
    HQ: int = 12  # q heads per core
    HKV: int = 2  # kv heads per core
    KB: int = 8  # k-tiles per projection SBUF-accumulation block
    CHUNK: int = 512  # tq chunk width (<= 512: one PSUM bank)
    sdt: str = "bf16"  # matmul operand dtype everywhere: "bf16" | "f32r"
    repeat: int = 1  # emit the whole body N times (timing amortization only)

    @property
    def SD(self):
        return mybir.dt.bfloat16 if self.sdt == "bf16" else mybir.dt.float32r

    @property
    def np_sd(self):
        if self.sdt == "bf16":
            import ml_dtypes
            return ml_dtypes.bfloat16
        return np.float32

    @property
    def KT(self):
        return self.D // 128

    @property
    def NT(self):
        return self.T // 128

    @property
    def NCH(self):
        return self.T // self.CHUNK

    @property
    def NPAT(self):
        return self.CHUNK // 128

    @property
    def NQD(self):
        return self.HQ * HD

    @property
    def NKD(self):
        return self.HKV * HD

    @property
    def GRP(self):
        return self.HQ // self.HKV

    @property
    def nKB(self):
        return self.KT // self.KB


FULL = Cfg()


def build_program(C: Cfg = FULL):
    nc = bacc.Bacc("TRN2", target_bir_lowering=False, debug=False)
    SD = C.SD

    xqT = nc.dram_tensor("xqT", [C.D, C.T], SD, kind="ExternalInput").ap()
    xkT = nc.dram_tensor("xkT", [C.D, C.T], SD, kind="ExternalInput").ap()
    xvT = nc.dram_tensor("xvT", [C.D, C.T], SD, kind="ExternalInput").ap()
    # wq regrouped host-side: [nKB, HQ, 128, KB*128]; each per-(block, head)
    # load is one fully contiguous DMA.
    wq_r = nc.dram_tensor("wq_r", [C.nKB, C.HQ, 128, C.KB * 128], SD,
                          kind="ExternalInput").ap()
    wk_r = nc.dram_tensor("wk_r", [C.KT, 128, C.NKD], SD, kind="ExternalInput").ap()
    wv_r = nc.dram_tensor("wv_r", [C.KT, 128, C.NKD], SD, kind="ExternalInput").ap()
    wo_g = nc.dram_tensor("wo_g", [C.NQD, C.D], SD, kind="ExternalInput").ap()
    cosT = nc.dram_tensor("cosT", [128, C.T], SD, kind="ExternalInput").ap()
    sinrT = nc.dram_tensor("sinrT", [128, C.T], SD, kind="ExternalInput").ap()
    # trimask[:, :128] = NEG (fully-masked block); [:, 128:] = diagonal
    # triangle (NEG above the diagonal); added to raw logits pre-tanh.
    trimask = nc.dram_tensor("trimask", [128, 256], SD, kind="ExternalInput").ap()
    bqh = nc.dram_tensor("bqh", [128, C.HQ], F32, kind="ExternalInput").ap()
    bkh = nc.dram_tensor("bkh", [128, C.HKV], F32, kind="ExternalInput").ap()
    ones_d = nc.dram_tensor("ones_d", [128, 1], SD, kind="ExternalInput").ap()
    ident_d = nc.dram_tensor("ident_d", [128, 128], SD, kind="ExternalInput").ap()
    rot_d = nc.dram_tensor("rot_d", [128, 128], SD, kind="ExternalInput").ap()
    out = nc.dram_tensor("out", [C.T, C.D], F32, kind="ExternalOutput").ap()
    out2 = nc.dram_tensor("out2", [C.T, C.D], F32, kind="ExternalOutput").ap()

    H2 = C.HQ // 2
    NCN = C.D // 512
    LKB = C.nKB - 1

    with tile.TileContext(nc) as tc:
        with ExitStack() as ctx:
            const = ctx.enter_context(tc.tile_pool(name="const", bufs=1))

            cos_sb = const.tile([128, C.T], SD, name="cos", tag="cos")
            sinr_sb = const.tile([128, C.T], SD, name="sinr", tag="sinr")
            tri_sb = const.tile([128, 256], SD, name="tri", tag="tri")
            ident_sb = const.tile([128, 128], SD, name="ident", tag="ident")
            rot_sb = const.tile([128, 128], SD, name="rot", tag="rot")
            bq_sb = const.tile([128, C.HQ], F32, name="bq", tag="bq")
            nc.sync.dma_start(bq_sb[:], bqh)
            bk_sb = const.tile([128, C.HKV], F32, name="bk", tag="bk")
            nc.sync.dma_start(bk_sb[:], bkh)

            zero_b = const.tile([128, 1], F32, name="zero_b", tag="zero_b")
            nc.vector.memset(zero_b[:], 0.0)
            negmax_b = const.tile([128, 1], F32, name="negmax_b", tag="negmax_b")
            nc.vector.memset(negmax_b[:], -MAXA)

            for _rep in range(C.repeat):
              with tc.tile_pool(name="resid", bufs=1) as resid, \
                   tc.tile_pool(name="rope", bufs=3) as rope_pool, \
                   tc.tile_pool(name="kvstream", bufs=6) as kvs, \
                   tc.tile_pool(name="xqstream", bufs=2) as xqs, \
                   tc.tile_pool(name="wqstream", bufs=4) as wqs, \
                   tc.tile_pool(name="attn_sb", bufs=4) as asb, \
                   tc.tile_pool(name="e_sb", bufs=6) as esb, \
                   tc.tile_pool(name="wostream", bufs=2) as wos, \
                   tc.tile_pool(name="obuf", bufs=4) as obp:
                kt_sb = [resid.tile([128, C.T], SD, name=f"kt{i}", tag=f"kt{i}") for i in range(C.HKV)]
                vt_sb = [resid.tile([128, C.T], SD, name=f"vt{i}", tag=f"vt{i}") for i in range(C.HKV)]
                v_sb = [resid.tile([128, C.NKD], SD, name=f"v{i}", tag=f"v{i}") for i in range(C.NT)]
                qt_sb = [resid.tile([128, C.T], SD, name=f"qt{h}", tag=f"qt{h}") for h in range(C.HQ)]
                ot_sb = [resid.tile([128, C.T], SD, name=f"ot{h}", tag=f"ot{h}")
                         for h in range(C.HQ)]

                def rope_inplace(pool, tag, x, c0, w):
                    """x[:, c0:c0+w] = x*cos + half_swap(x)*sinr in place; the
                    half swap is a PE matmul against the rot permutation."""
                    cs = slice(c0, c0 + w)
                    rp = pool.tile([128, w], F32, name="ropep", tag=tag)
                    nc.tensor.matmul(rp[:], rot_sb[:], x[:, cs], start=True, stop=True)
                    qrot = rope_pool.tile([128, w], SD, name="qrot", tag="qrot")
                    nc.vector.tensor_mul(out=qrot[:], in0=rp[:], in1=sinr_sb[:, cs])
                    nc.vector.tensor_mul(out=x[:, cs], in0=x[:, cs], in1=cos_sb[:, cs])
                    nc.vector.tensor_add(out=x[:, cs], in0=x[:, cs], in1=qrot[:])

                def q_head(kb, h, psum_pool, tag, wq_t=None, rope_now=True):
                    """Q projection of head h for block kb (direct emission)."""
                    if wq_t is None:
                        wq_t = wqs.tile([128, C.KB * 128], SD, name="wq", tag="wq")
                        nc.sync.dma_start(wq_t[:], wq_r[kb, h])
                    for c in range(C.NCH):
                        cs = slice(c * C.CHUNK, (c + 1) * C.CHUNK)
                        qp = psum_pool.tile([128, C.CHUNK], F32, name="qp", tag=tag)
                        for ki in range(C.KB):
                            nc.tensor.matmul(
                                qp[:], wq_t[:, ki * 128:(ki + 1) * 128],
                                xq_tiles[ki][:, cs],
                                start=(ki == 0), stop=(ki == C.KB - 1))
                        if kb == 0:
                            nc.scalar.activation(
                                qt_sb[h][:, cs], qp[:], AF.Identity,
                                bias=bq_sb[:, h:h + 1], scale=1.0)
                        else:
                            nc.vector.tensor_add(
                                out=qt_sb[h][:, cs], in0=qt_sb[h][:, cs],
                                in1=qp[:])
                        if kb == LKB and rope_now:
                            rope_inplace(psum_pool, tag, qt_sb[h],
                                         c * C.CHUNK, C.CHUNK)

                # ======== context A: projections ============================
                with tc.tile_pool(name="pps", bufs=8, space="PSUM") as pps:
                    for kb in range(C.nKB):
                        k0 = kb * C.KB
                        last = kb == LKB
                        xq_tiles = [
                            xqs.tile([128, C.T], SD, name=f"xq{i}", tag=f"xq{i}")
                            for i in range(C.KB)
                        ]
                        if kb == 0:
                            # block 0 is DMA-starved; Q's operands are the
                            # smallest, so load them and project Q first.
                            wq0 = []
                            for h in range(3):
                                t = wqs.tile([128, C.KB * 128], SD, name="wq",
                                             tag="wq")
                                nc.sync.dma_start(t[:], wq_r[0, h])
                                wq0.append(t)
                            for j, t in enumerate(xq_tiles):
                                nc.sync.dma_start(
                                    t[:], xqT[(k0 + j) * 128:(k0 + j + 1) * 128, :])
                            for h in range(C.HQ):
                                q_head(0, h, pps, "pp",
                                       wq_t=wq0[h] if h < 3 else None)
                        kp, vtp = {}, {}
                        for kv in range(C.HKV):
                            for c in range(C.NCH):
                                kp[kv, c] = pps.tile([128, C.CHUNK], F32,
                                                     name="kp", tag="pp")
                                vtp[kv, c] = pps.tile([128, C.CHUNK], F32,
                                                      name="vtp", tag="pp")
                        for i in range(C.KB):
                            k = k0 + i
                            xk_t = kvs.tile([128, C.T], SD, name="xk", tag="xk")
                            nc.sync.dma_start(xk_t[:], xkT[k * 128:(k + 1) * 128, :])
                            wk_t = kvs.tile([128, C.NKD], SD, name="wk", tag="wk")
                            nc.sync.dma_start(wk_t[:], wk_r[k])
                            xv_t = kvs.tile([128, C.T], SD, name="xv", tag="xv")
                            nc.sync.dma_start(xv_t[:], xvT[k * 128:(k + 1) * 128, :])
                            wv_t = kvs.tile([128, C.NKD], SD, name="wv", tag="wv")
                            nc.sync.dma_start(wv_t[:], wv_r[k])
                            if i == 1 and kb > 0:
                                for j, t in enumerate(xq_tiles):
                                    nc.sync.dma_start(
                                        t[:],
                                        xqT[(k0 + j) * 128:(k0 + j + 1) * 128, :])
                            if i == 3 and kb == 0 and _rep == 0:
                                nc.sync.dma_start(cos_sb[:], cosT)
                                nc.sync.dma_start(sinr_sb[:], sinrT)
                                nc.sync.dma_start(tri_sb[:], trimask)
                                nc.sync.dma_start(ident_sb[:], ident_d)
                                nc.sync.dma_start(rot_sb[:], rot_d)
                            for kv in range(C.HKV):
                                ks_ = slice(kv * 128, (kv + 1) * 128)
                                for c in range(C.NCH):
                                    cs = slice(c * C.CHUNK, (c + 1) * C.CHUNK)
                                    nc.tensor.matmul(
                                        kp[kv, c][:], wk_t[:, ks_], xk_t[:, cs],
                                        start=(i == 0), stop=(i == C.KB - 1))
                                    nc.tensor.matmul(
                                        vtp[kv, c][:], wv_t[:, ks_], xv_t[:, cs],
                                        start=(i == 0), stop=(i == C.KB - 1))
                        for kv in range(C.HKV):
                            for c in range(C.NCH):
                                cs = slice(c * C.CHUNK, (c + 1) * C.CHUNK)
                                if kb == 0:
                                    nc.scalar.activation(
                                        kt_sb[kv][:, cs], kp[kv, c][:], AF.Identity,
                                        bias=bk_sb[:, kv:kv + 1], scale=1.0)
                                    nc.scalar.activation(
                                        vt_sb[kv][:, cs], vtp[kv, c][:], AF.Copy)
                                else:
                                    nc.vector.tensor_add(
                                        out=kt_sb[kv][:, cs], in0=kt_sb[kv][:, cs],
                                        in1=kp[kv, c][:])
                                    nc.vector.tensor_add(
                                        out=vt_sb[kv][:, cs], in0=vt_sb[kv][:, cs],
                                        in1=vtp[kv, c][:])
                        if last:
                            # all K accumulate-adds are queued on DVE before
                            # the first rope matmul so the in-order PE doesn't
                            # stall on each add individually
                            for kv in range(C.HKV):
                                for c in range(C.NCH):
                                    rope_inplace(pps, "pp", kt_sb[kv],
                                                 c * C.CHUNK, C.CHUNK)
                            # Q of heads 0..H2-1 fills the PE while DVE works
                            # through the K/V accumulate + rope tail; each
                            # head's rope is emitted one head late so its DVE
                            # add has drained by then.
                            for h in range(H2):
                                q_head(LKB, h, pps, "pp", rope_now=False)
                                if h > 0:
                                    for c in range(C.NCH):
                                        rope_inplace(pps, "pp", qt_sb[h - 1],
                                                     c * C.CHUNK, C.CHUNK)
                            for c in range(C.NCH):
                                rope_inplace(pps, "pp", qt_sb[H2 - 1],
                                             c * C.CHUNK, C.CHUNK)
                            # V^T -> V natural via PE transposes
                            for kv in range(C.HKV):
                                for ti in range(C.NT):
                                    tp = pps.tile([128, 128], SD, name="vtr",
                                                  tag="pp")
                                    nc.tensor.transpose(
                                        tp[:],
                                        vt_sb[kv][:, ti * 128:(ti + 1) * 128],
                                        ident_sb[:])
                                    nc.scalar.activation(
                                        v_sb[ti][:, kv * 128:(kv + 1) * 128],
                                        tp[:], AF.Copy)
                        elif kb > 0:
                            for h in range(C.HQ):
                                q_head(kb, h, pps, "pp")

                # ======== context B: attention + O-proj (+ Q of h6..11) =====
                # PSUM tags: sp x1, qp x2 (Q accum + rope of late heads),
                # op x2, otp x2, rsum x1 -> 8 banks.
                from concourse import bass_isa
                with tc.tile_pool(name="qs", bufs=3, space="PSUM") as qsr, \
                     tc.tile_pool(name="qpp", bufs=1, space="PSUM") as qpp, \
                     tc.tile_pool(name="opp", bufs=2, space="PSUM") as opp, \
                     tc.tile_pool(name="otpp", bufs=2, space="PSUM") as otpp, \
                     tc.tile_pool(name="red_sb", bufs=2) as redp:

                    pending = []  # [gen, ns_per_yield] filler sources

                    def fill(target_ns):
                        while target_ns > 0 and pending:
                            gen, per = pending[0]
                            try:
                                next(gen)
                            except StopIteration:
                                pending.pop(0)
                                continue
                            target_ns -= per

                    def drain(gen):
                        while True:
                            try:
                                next(gen)
                            except StopIteration:
                                break

                    def attn_head(h, conly=None):
                        kv = h // C.GRP
                        for c in ((conly,) if conly is not None
                                  else range(C.NCH)):
                            cs0 = c * C.CHUNK
                            ntk = (c + 1) * C.NPAT
                            ot_p = otpp.tile([128, C.CHUNK], F32, name="otp",
                                             tag="otp")
                            e_acc = asb.tile([128, C.CHUNK], SD, name="eacc",
                                             tag="eacc")
                            for m in range(ntk):
                                a = m - c * C.NPAT
                                # diagonal-band tiles: trim columns to the
                                # causal support (bf16 matmuls run 1 cyc/row
                                # at any width, so the trim is exact)
                                off = 0 if a <= 0 else a * 128
                                el = slice(off, C.CHUNK)
                                ql = slice(cs0 + off, cs0 + C.CHUNK)
                                sp = qsr.tile([128, C.CHUNK], F32, name="sp",
                                              tag="qs")
                                band = a >= 0
                                nc.tensor.matmul(
                                    sp[:, el],
                                    kt_sb[kv][:, m * 128:(m + 1) * 128],
                                    qt_sb[h][:, ql], start=True, stop=not band)
                                if band:
                                    # accumulate the -1e9 mask into the raw
                                    # logits: identity x trimask slice; tanh
                                    # then saturates to -1, exp to ~0.
                                    ms = slice(off, (a + 1) * 128)
                                    tw = (a + 1) * 128 - off
                                    nc.tensor.matmul(
                                        sp[:, ms], ident_sb[:],
                                        tri_sb[:, 256 - tw:256],
                                        start=False, stop=True)
                                tca = asb.tile([128, C.CHUNK], F32, name="tc",
                                               tag="tc")
                                nc.scalar.activation(tca[:, el], sp[:, el], AF.Tanh,
                                                     bias=zero_b[:],
                                                     scale=MULT / MAXA)
                                e = esb.tile([128, C.CHUNK], SD, name="e", tag="e")
                                nc.scalar.activation(e[:, el], tca[:, el], AF.Exp,
                                                     scale=MAXA, bias=negmax_b[:])
                                # softmax denominator off the PE: e-tiles
                                # accumulate on DVE (pure-bf16 fast path);
                                # one Pool partition-sum per chunk at the end
                                if m == 0:
                                    nc.vector.tensor_copy(
                                        out=e_acc[:], in_=e[:])
                                else:
                                    nc.vector.tensor_add(
                                        out=e_acc[:, el], in0=e_acc[:, el],
                                        in1=e[:, el])
                                # PE filler emitted inside the tanh+exp wait
                                # window (first m-step pays the full chain)
                                fill(1800 if m == 0 else 1000)
                                nc.tensor.matmul(
                                    ot_p[:, el],
                                    v_sb[m][:, kv * 128:(kv + 1) * 128],
                                    e[:, el],
                                    start=(m == 0), stop=(m == ntk - 1))
                            red = redp.tile([128, C.CHUNK], F32, name="red",
                                            tag="red")
                            nc.gpsimd.partition_all_reduce(
                                red[:], e_acc[:], channels=128,
                                reduce_op=bass_isa.ReduceOp.add)
                            recip = asb.tile([1, C.CHUNK], F32, name="recip",
                                             tag="recip")
                            rscr = asb.tile([1, C.CHUNK], F32, name="rscr",
                                            tag="rscr")
                            nc.vector.reciprocal_approx_accurate(
                                out=recip[:], in_=red[0:1, :], scratch=rscr[:])
                            bc_sb = asb.tile([128, C.CHUNK], F32, name="bc_sb",
                                             tag="bc_sb")
                            nc.gpsimd.partition_broadcast(bc_sb[:], recip[:])
                            nc.vector.tensor_mul(
                                out=ot_sb[h][:, cs0:cs0 + C.CHUNK], in0=ot_p[:],
                                in1=bc_sb[:])

                    wq_pref = {}

                    def wq_prefetch(h):
                        if h < C.HQ and h not in wq_pref:
                            t = wqs.tile([128, C.KB * 128], SD, name="wq", tag="wq")
                            nc.sync.dma_start(t[:], wq_r[LKB, h])
                            wq_pref[h] = t

                    def q_units(h):
                        """Q-projection + rope of head h, chopped into ~3
                        matmul units so it can fill attention wait windows."""
                        wq_t = wq_pref.pop(h)
                        wq_prefetch(h + 2)
                        for c in range(C.NCH):
                            cs = slice(c * C.CHUNK, (c + 1) * C.CHUNK)
                            qp = qpp.tile([128, C.CHUNK], F32, name="qp", tag="qp")
                            for ki in range(C.KB):
                                nc.tensor.matmul(
                                    qp[:], wq_t[:, ki * 128:(ki + 1) * 128],
                                    xq_tiles[ki][:, cs],
                                    start=(ki == 0), stop=(ki == C.KB - 1))
                                if ki % 3 == 2:
                                    yield
                            nc.vector.tensor_add(
                                out=qt_sb[h][:, cs], in0=qt_sb[h][:, cs],
                                in1=qp[:])
                            yield
                            # rope after a consumer step so the DVE add above
                            # has drained (no in-order PE stall on the rope mm)
                            rope_inplace(qpp, "qp", qt_sb[h], c * C.CHUNK, C.CHUNK)
                            yield

                    def oproj_units(half, h0, nh, tis):
                        """Each next() emits one ti-unit (nh-matmul PSUM group
                        + copy + store). `tis` restricts the t-tiles so units
                        can unlock as soon as the needed attention chunks are
                        done (ti<4 only needs chunk 0)."""
                        dst = out if half == 0 else out2
                        for ncn in range(NCN):
                            ns = slice(ncn * 512, (ncn + 1) * 512)
                            wo_tiles = []
                            for j in range(nh):
                                t = wos.tile([128, 512], SD, name=f"wo{j}",
                                             tag=f"wo{j}")
                                nc.sync.dma_start(
                                    t[:],
                                    wo_g[(h0 + j) * 128:(h0 + j + 1) * 128, ns])
                                wo_tiles.append(t)
                            for ti in tis:
                                op = opp.tile([128, 512], F32, name="op", tag="op")
                                for j in range(nh):
                                    nc.tensor.matmul(
                                        op[:],
                                        ot_sb[h0 + j][:, ti * 128:(ti + 1) * 128],
                                        wo_tiles[j][:],
                                        start=(j == 0), stop=(j == nh - 1))
                                ob = obp.tile([128, 512], F32, name="ob", tag="ob")
                                nc.scalar.activation(ob[:], op[:], AF.Copy)
                                # store from the Act queue: emitted right after
                                # the copy on the same engine (no cross-engine
                                # wait, no SP-queue head-of-line blocking).
                                nc.scalar.dma_start(
                                    dst[ti * 128:(ti + 1) * 128, ns], ob[:])
                                yield

                    # chunk-major attention: after c0 of heads 0..5, the
                    # O-proj(half0) units for t-tiles 0..3 are already
                    # runnable and fill everything that follows. Q+rope of
                    # heads 6..11 run as fill generators during pass 1.
                    NPLO = C.NT // C.NCH  # t-tiles covered by chunk 0
                    qgens = []
                    wq_prefetch(H2)
                    wq_prefetch(H2 + 1)
                    for h in range(H2):
                        if h + H2 < C.HQ:
                            entry = [q_units(h + H2), 640]
                            qgens.append(entry)
                            nq = sum(1 for e_ in pending if e_[1] == 640)
                            pending.insert(nq, entry)
                        attn_head(h, 0)
                    pending.append(
                        [oproj_units(0, 0, H2, range(NPLO)), 1280])
                    for h in range(H2):
                        attn_head(h, 1)
                    # heads 6..11 Q must be complete before their attention
                    for entry in qgens:
                        drain(entry[0])
                        if entry in pending:
                            pending.remove(entry)
                    pending.append(
                        [oproj_units(0, 0, H2, range(NPLO, C.NT)), 1280])
                    for h in range(H2, C.HQ):
                        attn_head(h, 0)
                    pending.append(
                        [oproj_units(1, H2, C.HQ - H2, range(NPLO)), 1280])
                    for h in range(H2, C.HQ):
                        attn_head(h, 1)
                    pending.append(
                        [oproj_units(1, H2, C.HQ - H2, range(NPLO, C.NT)), 1280])
                    fill(10 ** 9)

    nc.compile()
    return nc


# ---------------------------------------------------------------------------
# Host side: sharding, rope tables, masks, gather.
# ---------------------------------------------------------------------------

def make_rope_tables(C: Cfg):
    exponents = np.arange(0, HD, 2, dtype=np.float32)
    inv_freq = (1.0 / (np.float32(ROPE_BASE) ** (exponents / np.float32(HD)))).astype(np.float32)
    t = np.arange(C.T, dtype=np.float32)
    phase = np.outer(t, inv_freq).astype(np.float32)  # [T, 64]
    phase = np.concatenate([phase, phase], axis=1)  # [T, 128]
    cosT = np.ascontiguousarray(np.cos(phase).astype(np.float32).T)  # [128, T]
    sinT = np.sin(phase).astype(np.float32).T  # [128, T]
    sinrT = sinT.copy()
    sinrT[0:64, :] *= -1.0  # sign of rotate-half folded into the table
    return cosT, np.ascontiguousarray(sinrT)


def make_trimask(C: Cfg, mask: np.ndarray):
    """[128, 256] additive mask: cols [0,128) fully masked (NEG); cols
    [128,256) the diagonal 128x128 triangle taken from the mask input.
    tri[p, 128+f] = 0 if mask[f, p] else NEG (local diagonal block)."""
    m2 = np.asarray(mask).reshape(mask.shape[-2], mask.shape[-1])
    diag = m2[:128, :128]  # [tq, tk] local diagonal block
    tri = np.where(diag.T, np.float32(0.0), np.float32(NEG))  # [tk(p), tq(f)]
    full = np.full((128, 128), np.float32(NEG), dtype=np.float32)
    return np.ascontiguousarray(np.concatenate([full, tri], axis=1))


def build_in_maps(C: Cfg, query, key, value, mask, wq, bq, wk, bk, wv, bv, wo,
                  n_model: int):
    sd = C.np_sd
    query = np.asarray(query, dtype=np.float32)
    key = np.asarray(key, dtype=np.float32)
    value = np.asarray(value, dtype=np.float32)
    wq = np.asarray(wq, dtype=np.float32)
    wk = np.asarray(wk, dtype=np.float32)
    wv = np.asarray(wv, dtype=np.float32)
    wo = np.asarray(wo, dtype=np.float32)
    bq = np.asarray(bq, dtype=np.float32)
    bk = np.asarray(bk, dtype=np.float32)

    B = query.shape[0]
    cosT, sinrT = make_rope_tables(C)
    trimask = make_trimask(C, mask)

    xT = {}
    for b in range(B):
        xT[b] = (
            np.ascontiguousarray(query[b].T).astype(sd),
            np.ascontiguousarray(key[b].T).astype(sd),
            np.ascontiguousarray(value[b].T).astype(sd),
        )
    gslices = {}
    for g in range(n_model):
        wq_g = wq[:, g * C.NQD:(g + 1) * C.NQD]
        # [nKB, HQ, 128, KB*128]: wq_r[kb, h, p, ki*128+c]
        #   = wq[(kb*KB+ki)*128 + p, h*128 + c]
        wq_r = np.ascontiguousarray(
            wq_g.reshape(C.nKB, C.KB, 128, C.HQ, 128)
            .transpose(0, 3, 2, 1, 4)
            .reshape(C.nKB, C.HQ, 128, C.KB * 128)).astype(sd)
        wk_r = np.ascontiguousarray(
            wk[:, g * C.NKD:(g + 1) * C.NKD].reshape(C.KT, 128, C.NKD)).astype(sd)
        wv_r = np.ascontiguousarray(
            wv[:, g * C.NKD:(g + 1) * C.NKD].reshape(C.KT, 128, C.NKD)).astype(sd)
        wo_gs = np.ascontiguousarray(wo[g * C.NQD:(g + 1) * C.NQD, :]).astype(sd)
        bqh = np.ascontiguousarray(bq[g * C.NQD:(g + 1) * C.NQD].reshape(C.HQ, 128).T)
        bkh = np.ascontiguousarray(bk[g * C.NKD:(g + 1) * C.NKD].reshape(C.HKV, 128).T)
        gslices[g] = (wq_r, wk_r, wv_r, wo_gs, bqh, bkh)

    in_maps = []
    for core in range(B * n_model):
        b, g = divmod(core, n_model)
        wq_r, wk_r, wv_r, wo_gs, bqh, bkh = gslices[g]
        in_maps.append({
            "xqT": xT[b][0], "xkT": xT[b][1], "xvT": xT[b][2],
            "wq_r": wq_r, "wk_r": wk_r, "wv_r": wv_r, "wo_g": wo_gs,
            "cosT": cosT.astype(sd), "sinrT": sinrT.astype(sd),
            "trimask": trimask.astype(sd),
            "bqh": bqh, "bkh": bkh,
            "ones_d": np.ones((128, 1), dtype=sd),
            "ident_d": np.eye(128, dtype=np.float32).astype(sd),
            # rotate-half permutation: out[i] = x[(i+64)%128]
            "rot_d": np.ascontiguousarray(
                np.roll(np.eye(128, dtype=np.float32), 64, axis=0)).astype(sd),
        })
    return in_maps


def assemble_output(C: Cfg, results, B, n_model, bv, wo):
    D = C.D
    out = np.zeros((B, C.T, D), dtype=np.float32)
    for core in range(B * n_model):
        b, g = divmod(core, n_model)
        out[b] += results[core]["out"]
        out[b] += results[core]["out2"]
    # bias_v enters linearly: rows of normalized attn weights sum to 1, so
    # O = P@V + 1*bv_exp^T exactly; fold the rank-1 term through wo on host.
    bv = np.asarray(bv, dtype=np.float32)
    wo = np.asarray(wo, dtype=np.float32)
    if np.any(bv):
        corr = np.zeros((D,), dtype=np.float32)
        for g in range(n_model):
            bv_g = bv[g * C.NKD:(g + 1) * C.NKD]
            bvexp = np.empty((C.NQD,), dtype=np.float32)
            for h in range(C.HQ):
                kvl = h // C.GRP
                bvexp[h * 128:(h + 1) * 128] = bv_g[kvl * 128:(kvl + 1) * 128]
            corr += bvexp @ wo[g * C.NQD:(g + 1) * C.NQD, :]
        out += corr[None, None, :]
    return out


_PROG_CACHE = {}


def get_program(C: Cfg = FULL):
    key = C
    if key not in _PROG_CACHE:
        _PROG_CACHE[key] = build_program(C)
    return _PROG_CACHE[key]


def kernel(query, key, value, mask, wq, bq, wk, bk, wv, bv, wo):
    C = FULL
    B = query.shape[0]
    n_model = (wq.shape[1] // HD) // C.HQ
    n_cores = B * n_model
    nc = get_program(C)
    in_maps = build_in_maps(C, query, key, value, mask, wq, bq, wk, bk, wv, bv, wo,
                            n_model)
    res = bass_utils.run_bass_kernel_spmd(nc, in_maps, core_ids=list(range(n_cores)))
    return assemble_output(C, res.results, B, n_model, bv, wo)


# revision 65
# speedup vs baseline: 1.7889x; 1.7889x over previous
"""Trainium2 Bass kernel: GQA multi-head attention block (nn_MultiHeadAttention).

Full-input contract: kernel(**inputs) takes the unsharded inputs and returns the
full [B, T, D] output. Internally shards across 8 NeuronCores as
2 (batch / data axis) x 4 (head groups / model axis): each core processes one
batch element and 12 q heads (2 kv heads) including the row-shard of the output
projection; the host sums the 4 model-parallel partial outputs per batch.

Per-core compute layout ("transposed attention"):
  - host passes x^T [D, T] so projections emit Q^T/K^T [d, t] directly
    (features on partitions).
  - S^T tile [tk=128, tq<=512] = single matmul (contraction d=128).
  - soft logit cap: 30*tanh(logits/30); softmax uses the fixed max 30
    (tanh bounds logits to [-30,30], so no row-max pass is needed).
  - causal: upper-triangular tiles are skipped structurally; diagonal-band
    tiles are column-trimmed to the causal support (kept >=256 wide); the
    -1e9 mask is ACCUMULATED INTO THE LOGIT PSUM by a second matmul
    (identity x trimask) so no vector-engine op sits in the softmax chain
    (tanh saturates to -1, exp maps it to ~0 -- exact).
  - rope: rotate-half is a PE matmul against a [128,128] permutation matrix;
    the sign lives in the sin table; the elementwise combine runs on DVE in
    bf16 (2-byte fast path).
  - softmax denominator via ones-column matmul (PE), 1/r broadcast across
    partitions on the Pool engine.
  - every matmul operand is bf16 (streams, weights, Q/K/V/P/ot, constants);
    PSUM accumulation and the softmax chain stay f32.

Schedule (PE is in-order, so filler work is EMITTED inside the tanh+exp wait
window of each attention m-step):
  - block 0 projects Q before K/V (it is DMA-starved; Q's operands arrive
    first); blocks 1..nKB-1 run K/V then Q for all heads.
  - last block (context A): K/V + K-rope + Q of heads 0..5 + V-transposes.
  - context B: per head h: attention; Q+rope of heads 6..11 and (from head 5)
    O-projection ti-units are generators, drained one unit per wait window.
  - output stores ride the Act engine's DGE queue; SP carries only loads.
"""

import sys
from contextlib import ExitStack
from dataclasses import dataclass

for _p in (
    "/opt/trn_rl_repo",
    "/opt/pypackages",
    "/root/.axon_site/_ro/trn_rl_repo",
    "/root/.axon_site/_ro/pypackages",
):
    if _p not in sys.path:
        sys.path.insert(0, _p)

import numpy as np  # noqa: E402

import concourse.mybir as mybir  # noqa: E402
import concourse.tile as tile  # noqa: E402
from concourse import bacc, bass_utils  # noqa: E402

MULT = 0.08838834764831845  # 1/sqrt(128)
MAXA = 30.0  # tanh logit cap
NEG = -1.0e9  # additive logit mask
ROPE_BASE = 10000.0
HD = 128  # head dim (fixed: rope halves assume 64/64)

F32 = mybir.dt.float32
AF = mybir.ActivationFunctionType


@dataclass(frozen=True)
class Cfg:
    T: int = 1024  # tokens per core
    D: int = 6144  # model dim
    HQ: int = 12  # q heads per core
    HKV: int = 2  # kv heads per core
    KB: int = 8  # k-tiles per projection SBUF-accumulation block
    CHUNK: int = 512  # tq chunk width (<= 512: one PSUM bank)
    sdt: str = "bf16"  # matmul operand dtype everywhere: "bf16" | "f32r"
    repeat: int = 1  # emit the whole body N times (timing amortization only)

    @property
    def SD(self):
        return mybir.dt.bfloat16 if self.sdt == "bf16" else mybir.dt.float32r

    @property
    def np_sd(self):
        if self.sdt == "bf16":
            import ml_dtypes
            return ml_dtypes.bfloat16
        return np.float32

    @property
    def KT(self):
        return self.D // 128

    @property
    def NT(self):
        return self.T // 128

    @property
    def NCH(self):
        return self.T // self.CHUNK

    @property
    def NPAT(self):
        return self.CHUNK // 128

    @property
    def NQD(self):
        return self.HQ * HD

    @property
    def NKD(self):
        return self.HKV * HD

    @property
    def GRP(self):
        return self.HQ // self.HKV

    @property
    def nKB(self):
        return self.KT // self.KB


FULL = Cfg()


def build_program(C: Cfg = FULL):
    nc = bacc.Bacc("TRN2", target_bir_lowering=False, debug=False)
    SD = C.SD

    xqT = nc.dram_tensor("xqT", [C.D, C.T], SD, kind="ExternalInput").ap()
    xkT = nc.dram_tensor("xkT", [C.D, C.T], SD, kind="ExternalInput").ap()
    xvT = nc.dram_tensor("xvT", [C.D, C.T], SD, kind="ExternalInput").ap()
    # wq regrouped host-side: [nKB, HQ, 128, KB*128]; each per-(block, head)
    # load is one fully contiguous DMA.
    wq_r = nc.dram_tensor("wq_r", [C.nKB, C.HQ, 128, C.KB * 128], SD,
                          kind="ExternalInput").ap()
    wk_r = nc.dram_tensor("wk_r", [C.KT, 128, C.NKD], SD, kind="ExternalInput").ap()
    wv_r = nc.dram_tensor("wv_r", [C.KT, 128, C.NKD], SD, kind="ExternalInput").ap()
    wo_g = nc.dram_tensor("wo_g", [C.NQD, C.D], SD, kind="ExternalInput").ap()
    cosT = nc.dram_tensor("cosT", [128, C.T], SD, kind="ExternalInput").ap()
    sinrT = nc.dram_tensor("sinrT", [128, C.T], SD, kind="ExternalInput").ap()
    # trimask[:, :128] = NEG (fully-masked block); [:, 128:] = diagonal
    # triangle (NEG above the diagonal); added to raw logits pre-tanh.
    trimask = nc.dram_tensor("trimask", [128, 256], SD, kind="ExternalInput").ap()
    bqh = nc.dram_tensor("bqh", [128, C.HQ], F32, kind="ExternalInput").ap()
    bkh = nc.dram_tensor("bkh", [128, C.HKV], F32, kind="ExternalInput").ap()
    ones_d = nc.dram_tensor("ones_d", [128, 1], SD, kind="ExternalInput").ap()
    ident_d = nc.dram_tensor("ident_d", [128, 128], SD, kind="ExternalInput").ap()
    rot_d = nc.dram_tensor("rot_d", [128, 128], SD, kind="ExternalInput").ap()
    out = nc.dram_tensor("out", [C.T, C.D], F32, kind="ExternalOutput").ap()
    out2 = nc.dram_tensor("out2", [C.T, C.D], F32, kind="ExternalOutput").ap()

    H2 = C.HQ // 2
    NCN = C.D // 512
    LKB = C.nKB - 1

    with tile.TileContext(nc) as tc:
        with ExitStack() as ctx:
            const = ctx.enter_context(tc.tile_pool(name="const", bufs=1))

            cos_sb = const.tile([128, C.T], SD, name="cos", tag="cos")
            sinr_sb = const.tile([128, C.T], SD, name="sinr", tag="sinr")
            tri_sb = const.tile([128, 256], SD, name="tri", tag="tri")
            ident_sb = const.tile([128, 128], SD, name="ident", tag="ident")
            rot_sb = const.tile([128, 128], SD, name="rot", tag="rot")
            bq_sb = const.tile([128, C.HQ], F32, name="bq", tag="bq")
            nc.sync.dma_start(bq_sb[:], bqh)
            bk_sb = const.tile([128, C.HKV], F32, name="bk", tag="bk")
            nc.sync.dma_start(bk_sb[:], bkh)

            zero_b = const.tile([128, 1], F32, name="zero_b", tag="zero_b")
            nc.vector.memset(zero_b[:], 0.0)
            negmax_b = const.tile([128, 1], F32, name="negmax_b", tag="negmax_b")
            nc.vector.memset(negmax_b[:], -MAXA)

            for _rep in range(C.repeat):
              with tc.tile_pool(name="resid", bufs=1) as resid, \
                   tc.tile_pool(name="rope", bufs=3) as rope_pool, \
                   tc.tile_pool(name="kvstream", bufs=6) as kvs, \
                   tc.tile_pool(name="xqstream", bufs=2) as xqs, \
                   tc.tile_pool(name="wqstream", bufs=4) as wqs, \
                   tc.tile_pool(name="attn_sb", bufs=4) as asb, \
                   tc.tile_pool(name="e_sb", bufs=6) as esb, \
                   tc.tile_pool(name="wostream", bufs=2) as wos, \
                   tc.tile_pool(name="obuf", bufs=4) as obp:
                kt_sb = [resid.tile([128, C.T], SD, name=f"kt{i}", tag=f"kt{i}") for i in range(C.HKV)]
                vt_sb = [resid.tile([128, C.T], SD, name=f"vt{i}", tag=f"vt{i}") for i in range(C.HKV)]
                v_sb = [resid.tile([128, C.NKD], SD, name=f"v{i}", tag=f"v{i}") for i in range(C.NT)]
                qt_sb = [resid.tile([128, C.T], SD, name=f"qt{h}", tag=f"qt{h}") for h in range(C.HQ)]
                ot_sb = [resid.tile([128, C.T], SD, name=f"ot{h}", tag=f"ot{h}")
                         for h in range(C.HQ)]

                def rope_inplace(pool, tag, x, c0, w):
                    """x[:, c0:c0+w] = x*cos + half_swap(x)*sinr in place; the
                    half swap is a PE matmul against the rot permutation."""
                    cs = slice(c0, c0 + w)
                    rp = pool.tile([128, w], F32, name="ropep", tag=tag)
                    nc.tensor.matmul(rp[:], rot_sb[:], x[:, cs], start=True, stop=True)
                    qrot = rope_pool.tile([128, w], SD, name="qrot", tag="qrot")
                    nc.vector.tensor_mul(out=qrot[:], in0=rp[:], in1=sinr_sb[:, cs])
                    nc.vector.tensor_mul(out=x[:, cs], in0=x[:, cs], in1=cos_sb[:, cs])
                    nc.vector.tensor_add(out=x[:, cs], in0=x[:, cs], in1=qrot[:])

                def q_head(kb, h, psum_pool, tag, wq_t=None, rope_now=True):
                    """Q projection of head h for block kb (direct emission)."""
                    if wq_t is None:
                        wq_t = wqs.tile([128, C.KB * 128], SD, name="wq", tag="wq")
                        nc.sync.dma_start(wq_t[:], wq_r[kb, h])
                    for c in range(C.NCH):
                        cs = slice(c * C.CHUNK, (c + 1) * C.CHUNK)
                        qp = psum_pool.tile([128, C.CHUNK], F32, name="qp", tag=tag)
                        for ki in range(C.KB):
                            nc.tensor.matmul(
                                qp[:], wq_t[:, ki * 128:(ki + 1) * 128],
                                xq_tiles[ki][:, cs],
                                start=(ki == 0), stop=(ki == C.KB - 1))
                        if kb == 0:
                            nc.scalar.activation(
                                qt_sb[h][:, cs], qp[:], AF.Identity,
                                bias=bq_sb[:, h:h + 1], scale=1.0)
                        else:
                            nc.vector.tensor_add(
                                out=qt_sb[h][:, cs], in0=qt_sb[h][:, cs],
                                in1=qp[:])
                        if kb == LKB and rope_now:
                            rope_inplace(psum_pool, tag, qt_sb[h],
                                         c * C.CHUNK, C.CHUNK)

                # ======== context A: projections ============================
                with tc.tile_pool(name="pps", bufs=8, space="PSUM") as pps:
                    for kb in range(C.nKB):
                        k0 = kb * C.KB
                        last = kb == LKB
                        xq_tiles = [
                            xqs.tile([128, C.T], SD, name=f"xq{i}", tag=f"xq{i}")
                            for i in range(C.KB)
                        ]
                        if kb == 0:
                            # block 0 is DMA-starved; Q's operands are the
                            # smallest, so load them and project Q first.
                            wq0 = []
                            for h in range(3):
                                t = wqs.tile([128, C.KB * 128], SD, name="wq",
                                             tag="wq")
                                nc.sync.dma_start(t[:], wq_r[0, h])
                                wq0.append(t)
                            for j, t in enumerate(xq_tiles):
                                nc.sync.dma_start(
                                    t[:], xqT[(k0 + j) * 128:(k0 + j + 1) * 128, :])
                            for h in range(C.HQ):
                                q_head(0, h, pps, "pp",
                                       wq_t=wq0[h] if h < 3 else None)
                        kp, vtp = {}, {}
                        for kv in range(C.HKV):
                            for c in range(C.NCH):
                                kp[kv, c] = pps.tile([128, C.CHUNK], F32,
                                                     name="kp", tag="pp")
                                vtp[kv, c] = pps.tile([128, C.CHUNK], F32,
                                                      name="vtp", tag="pp")
                        for i in range(C.KB):
                            k = k0 + i
                            xk_t = kvs.tile([128, C.T], SD, name="xk", tag="xk")
                            nc.sync.dma_start(xk_t[:], xkT[k * 128:(k + 1) * 128, :])
                            wk_t = kvs.tile([128, C.NKD], SD, name="wk", tag="wk")
                            nc.sync.dma_start(wk_t[:], wk_r[k])
                            xv_t = kvs.tile([128, C.T], SD, name="xv", tag="xv")
                            nc.sync.dma_start(xv_t[:], xvT[k * 128:(k + 1) * 128, :])
                            wv_t = kvs.tile([128, C.NKD], SD, name="wv", tag="wv")
                            nc.sync.dma_start(wv_t[:], wv_r[k])
                            if i == 1 and kb > 0:
                                for j, t in enumerate(xq_tiles):
                                    nc.sync.dma_start(
                                        t[:],
                                        xqT[(k0 + j) * 128:(k0 + j + 1) * 128, :])
                            if i == 3 and kb == 0 and _rep == 0:
                                nc.sync.dma_start(cos_sb[:], cosT)
                                nc.sync.dma_start(sinr_sb[:], sinrT)
                                nc.sync.dma_start(tri_sb[:], trimask)
                                nc.sync.dma_start(ident_sb[:], ident_d)
                                nc.sync.dma_start(rot_sb[:], rot_d)
                            for kv in range(C.HKV):
                                ks_ = slice(kv * 128, (kv + 1) * 128)
                                for c in range(C.NCH):
                                    cs = slice(c * C.CHUNK, (c + 1) * C.CHUNK)
                                    nc.tensor.matmul(
                                        kp[kv, c][:], wk_t[:, ks_], xk_t[:, cs],
                                        start=(i == 0), stop=(i == C.KB - 1))
                                    nc.tensor.matmul(
                                        vtp[kv, c][:], wv_t[:, ks_], xv_t[:, cs],
                                        start=(i == 0), stop=(i == C.KB - 1))
                        for kv in range(C.HKV):
                            for c in range(C.NCH):
                                cs = slice(c * C.CHUNK, (c + 1) * C.CHUNK)
                                if kb == 0:
                                    nc.scalar.activation(
                                        kt_sb[kv][:, cs], kp[kv, c][:], AF.Identity,
                                        bias=bk_sb[:, kv:kv + 1], scale=1.0)
                                    nc.scalar.activation(
                                        vt_sb[kv][:, cs], vtp[kv, c][:], AF.Copy)
                                else:
                                    nc.vector.tensor_add(
                                        out=kt_sb[kv][:, cs], in0=kt_sb[kv][:, cs],
                                        in1=kp[kv, c][:])
                                    nc.vector.tensor_add(
                                        out=vt_sb[kv][:, cs], in0=vt_sb[kv][:, cs],
                                        in1=vtp[kv, c][:])
                        if last:
                            # all K accumulate-adds are queued on DVE before
                            # the first rope matmul so the in-order PE doesn't
                            # stall on each add individually
                            for kv in range(C.HKV):
                                for c in range(C.NCH):
                                    rope_inplace(pps, "pp", kt_sb[kv],
                                                 c * C.CHUNK, C.CHUNK)
                            # Q of heads 0..H2-1 fills the PE while DVE works
                            # through the K/V accumulate + rope tail; each
                            # head's rope is emitted one head late so its DVE
                            # add has drained by then.
                            for h in range(H2):
                                q_head(LKB, h, pps, "pp", rope_now=False)
                                if h > 0:
                                    for c in range(C.NCH):
                                        rope_inplace(pps, "pp", qt_sb[h - 1],
                                                     c * C.CHUNK, C.CHUNK)
                            for c in range(C.NCH):
                                rope_inplace(pps, "pp", qt_sb[H2 - 1],
                                             c * C.CHUNK, C.CHUNK)
                            # V^T -> V natural via PE transposes
                            for kv in range(C.HKV):
                                for ti in range(C.NT):
                                    tp = pps.tile([128, 128], SD, name="vtr",
                                                  tag="pp")
                                    nc.tensor.transpose(
                                        tp[:],
                                        vt_sb[kv][:, ti * 128:(ti + 1) * 128],
                                        ident_sb[:])
                                    nc.scalar.activation(
                                        v_sb[ti][:, kv * 128:(kv + 1) * 128],
                                        tp[:], AF.Copy)
                        elif kb > 0:
                            for h in range(C.HQ):
                                q_head(kb, h, pps, "pp")

                # ======== context B: attention + O-proj (+ Q of h6..11) =====
                # PSUM tags: sp x1, qp x2 (Q accum + rope of late heads),
                # op x2, otp x2, rsum x1 -> 8 banks.
                from concourse import bass_isa
                with tc.tile_pool(name="qs", bufs=3, space="PSUM") as qsr, \
                     tc.tile_pool(name="qpp", bufs=1, space="PSUM") as qpp, \
                     tc.tile_pool(name="opp", bufs=2, space="PSUM") as opp, \
                     tc.tile_pool(name="otpp", bufs=2, space="PSUM") as otpp, \
                     tc.tile_pool(name="red_sb", bufs=2) as redp:

                    pending = []  # [gen, ns_per_yield] filler sources

                    def fill(target_ns):
                        while target_ns > 0 and pending:
                            gen, per = pending[0]
                            try:
                                next(gen)
                            except StopIteration:
                                pending.pop(0)
                                continue
                            target_ns -= per

                    def drain(gen):
                        while True:
                            try:
                                next(gen)
                            except StopIteration:
                                break

                    def attn_head(h, conly=None):
                        kv = h // C.GRP
                        for c in ((conly,) if conly is not None
                                  else range(C.NCH)):
                            cs0 = c * C.CHUNK
                            ntk = (c + 1) * C.NPAT
                            ot_p = otpp.tile([128, C.CHUNK], F32, name="otp",
                                             tag="otp")
                            e_acc = asb.tile([128, C.CHUNK], SD, name="eacc",
                                             tag="eacc")
                            for m in range(ntk):
                                a = m - c * C.NPAT
                                # diagonal-band tiles: trim columns to the
                                # causal support (bf16 matmuls run 1 cyc/row
                                # at any width, so the trim is exact)
                                off = 0 if a <= 0 else a * 128
                                el = slice(off, C.CHUNK)
                                ql = slice(cs0 + off, cs0 + C.CHUNK)
                                sp = qsr.tile([128, C.CHUNK], F32, name="sp",
                                              tag="qs")
                                band = a >= 0
                                nc.tensor.matmul(
                                    sp[:, el],
                                    kt_sb[kv][:, m * 128:(m + 1) * 128],
                                    qt_sb[h][:, ql], start=True, stop=not band)
                                if band:
                                    # accumulate the -1e9 mask into the raw
                                    # logits: identity x trimask slice; tanh
                                    # then saturates to -1, exp to ~0.
                                    ms = slice(off, (a + 1) * 128)
                                    tw = (a + 1) * 128 - off
                                    nc.tensor.matmul(
                                        sp[:, ms], ident_sb[:],
                                        tri_sb[:, 256 - tw:256],
                                        start=False, stop=True)
                                tca = asb.tile([128, C.CHUNK], F32, name="tc",
                                               tag="tc")
                                nc.scalar.activation(tca[:, el], sp[:, el], AF.Tanh,
                                                     bias=zero_b[:],
                                                     scale=MULT / MAXA)
                                e = esb.tile([128, C.CHUNK], SD, name="e", tag="e")
                                nc.scalar.activation(e[:, el], tca[:, el], AF.Exp,
                                                     scale=MAXA, bias=negmax_b[:])
                                # softmax denominator off the PE: e-tiles
                                # accumulate on DVE (pure-bf16 fast path);
                                # one Pool partition-sum per chunk at the end
                                if m == 0:
                                    nc.vector.tensor_copy(
                                        out=e_acc[:], in_=e[:])
                                else:
                                    nc.vector.tensor_add(
                                        out=e_acc[:, el], in0=e_acc[:, el],
                                        in1=e[:, el])
                                # PE filler emitted inside the tanh+exp wait
                                # window (first m-step pays the full chain)
                                fill(1800 if m == 0 else 1000)
                                nc.tensor.matmul(
                                    ot_p[:, el],
                                    v_sb[m][:, kv * 128:(kv + 1) * 128],
                                    e[:, el],
                                    start=(m == 0), stop=(m == ntk - 1))
                            red = redp.tile([128, C.CHUNK], F32, name="red",
                                            tag="red")
                            nc.gpsimd.partition_all_reduce(
                                red[:], e_acc[:], channels=128,
                                reduce_op=bass_isa.ReduceOp.add)
                            recip = asb.tile([1, C.CHUNK], F32, name="recip",
                                             tag="recip")
                            rscr = asb.tile([1, C.CHUNK], F32, name="rscr",
                                            tag="rscr")
                            nc.vector.reciprocal_approx_accurate(
                                out=recip[:], in_=red[0:1, :], scratch=rscr[:])
                            bc_sb = asb.tile([128, C.CHUNK], F32, name="bc_sb",
                                             tag="bc_sb")
                            nc.gpsimd.partition_broadcast(bc_sb[:], recip[:])
                            nc.vector.tensor_mul(
                                out=ot_sb[h][:, cs0:cs0 + C.CHUNK], in0=ot_p[:],
                                in1=bc_sb[:])

                    wq_pref = {}

                    def wq_prefetch(h):
                        if h < C.HQ and h not in wq_pref:
                            t = wqs.tile([128, C.KB * 128], SD, name="wq", tag="wq")
                            nc.sync.dma_start(t[:], wq_r[LKB, h])
                            wq_pref[h] = t

                    def q_units(h):
                        """Q-projection + rope of head h, chopped into ~3
                        matmul units so it can fill attention wait windows."""
                        wq_t = wq_pref.pop(h)
                        wq_prefetch(h + 2)
                        for c in range(C.NCH):
                            cs = slice(c * C.CHUNK, (c + 1) * C.CHUNK)
                            qp = qpp.tile([128, C.CHUNK], F32, name="qp", tag="qp")
                            for ki in range(C.KB):
                                nc.tensor.matmul(
                                    qp[:], wq_t[:, ki * 128:(ki + 1) * 128],
                                    xq_tiles[ki][:, cs],
                                    start=(ki == 0), stop=(ki == C.KB - 1))
                                if ki % 3 == 2:
                                    yield
                            nc.vector.tensor_add(
                                out=qt_sb[h][:, cs], in0=qt_sb[h][:, cs],
                                in1=qp[:])
                            yield
                            # rope after a consumer step so the DVE add above
                            # has drained (no in-order PE stall on the rope mm)
                            rope_inplace(qpp, "qp", qt_sb[h], c * C.CHUNK, C.CHUNK)
                            yield

                    def oproj_units(half, h0, nh, tis):
                        """Each next() emits one ti-unit (nh-matmul PSUM group
                        + copy + store). `tis` restricts the t-tiles so units
                        can unlock as soon as the needed attention chunks are
                        done (ti<4 only needs chunk 0)."""
                        dst = out if half == 0 else out2
                        for ncn in range(NCN):
                            ns = slice(ncn * 512, (ncn + 1) * 512)
                            wo_tiles = []
                            for j in range(nh):
                                t = wos.tile([128, 512], SD, name=f"wo{j}",
                                             tag=f"wo{j}")
                                nc.sync.dma_start(
                                    t[:],
                                    wo_g[(h0 + j) * 128:(h0 + j + 1) * 128, ns])
                                wo_tiles.append(t)
                            for ti in tis:
                                op = opp.tile([128, 512], F32, name="op", tag="op")
                                for j in range(nh):
                                    nc.tensor.matmul(
                                        op[:],
                                        ot_sb[h0 + j][:, ti * 128:(ti + 1) * 128],
                                        wo_tiles[j][:],
                                        start=(j == 0), stop=(j == nh - 1))
                                ob = obp.tile([128, 512], F32, name="ob", tag="ob")
                                # copy on DVE so the op-bank free doesn't queue
                                # behind tanh/exp on the Act engine; store on
                                # SP (its queue only has well-prefetched loads)
                                nc.vector.tensor_copy(out=ob[:], in_=op[:])
                                nc.sync.dma_start(
                                    dst[ti * 128:(ti + 1) * 128, ns], ob[:])
                                yield

                    # chunk-major attention: after c0 of heads 0..5, the
                    # O-proj(half0) units for t-tiles 0..3 are already
                    # runnable and fill everything that follows. Q+rope of
                    # heads 6..11 run as fill generators during pass 1.
                    NPLO = C.NT // C.NCH  # t-tiles covered by chunk 0
                    qgens = []
                    wq_prefetch(H2)
                    wq_prefetch(H2 + 1)
                    for h in range(H2):
                        if h + H2 < C.HQ:
                            entry = [q_units(h + H2), 640]
                            qgens.append(entry)
                            nq = sum(1 for e_ in pending if e_[1] == 640)
                            pending.insert(nq, entry)
                        attn_head(h, 0)
                    pending.append(
                        [oproj_units(0, 0, H2, range(NPLO)), 1280])
                    for h in range(H2):
                        attn_head(h, 1)
                    # heads 6..11 Q must be complete before their attention
                    for entry in qgens:
                        drain(entry[0])
                        if entry in pending:
                            pending.remove(entry)
                    pending.append(
                        [oproj_units(0, 0, H2, range(NPLO, C.NT)), 1280])
                    for h in range(H2, C.HQ):
                        attn_head(h, 0)
                    pending.append(
                        [oproj_units(1, H2, C.HQ - H2, range(NPLO)), 1280])
                    for h in range(H2, C.HQ):
                        attn_head(h, 1)
                    pending.append(
                        [oproj_units(1, H2, C.HQ - H2, range(NPLO, C.NT)), 1280])
                    fill(10 ** 9)

    nc.compile()
    return nc


# ---------------------------------------------------------------------------
# Host side: sharding, rope tables, masks, gather.
# ---------------------------------------------------------------------------

def make_rope_tables(C: Cfg):
    exponents = np.arange(0, HD, 2, dtype=np.float32)
    inv_freq = (1.0 / (np.float32(ROPE_BASE) ** (exponents / np.float32(HD)))).astype(np.float32)
    t = np.arange(C.T, dtype=np.float32)
    phase = np.outer(t, inv_freq).astype(np.float32)  # [T, 64]
    phase = np.concatenate([phase, phase], axis=1)  # [T, 128]
    cosT = np.ascontiguousarray(np.cos(phase).astype(np.float32).T)  # [128, T]
    sinT = np.sin(phase).astype(np.float32).T  # [128, T]
    sinrT = sinT.copy()
    sinrT[0:64, :] *= -1.0  # sign of rotate-half folded into the table
    return cosT, np.ascontiguousarray(sinrT)


def make_trimask(C: Cfg, mask: np.ndarray):
    """[128, 256] additive mask: cols [0,128) fully masked (NEG); cols
    [128,256) the diagonal 128x128 triangle taken from the mask input.
    tri[p, 128+f] = 0 if mask[f, p] else NEG (local diagonal block)."""
    m2 = np.asarray(mask).reshape(mask.shape[-2], mask.shape[-1])
    diag = m2[:128, :128]  # [tq, tk] local diagonal block
    tri = np.where(diag.T, np.float32(0.0), np.float32(NEG))  # [tk(p), tq(f)]
    full = np.full((128, 128), np.float32(NEG), dtype=np.float32)
    return np.ascontiguousarray(np.concatenate([full, tri], axis=1))


def build_in_maps(C: Cfg, query, key, value, mask, wq, bq, wk, bk, wv, bv, wo,
                  n_model: int):
    sd = C.np_sd
    query = np.asarray(query, dtype=np.float32)
    key = np.asarray(key, dtype=np.float32)
    value = np.asarray(value, dtype=np.float32)
    wq = np.asarray(wq, dtype=np.float32)
    wk = np.asarray(wk, dtype=np.float32)
    wv = np.asarray(wv, dtype=np.float32)
    wo = np.asarray(wo, dtype=np.float32)
    bq = np.asarray(bq, dtype=np.float32)
    bk = np.asarray(bk, dtype=np.float32)

    B = query.shape[0]
    cosT, sinrT = make_rope_tables(C)
    trimask = make_trimask(C, mask)

    xT = {}
    for b in range(B):
        xT[b] = (
            np.ascontiguousarray(query[b].T).astype(sd),
            np.ascontiguousarray(key[b].T).astype(sd),
            np.ascontiguousarray(value[b].T).astype(sd),
        )
    gslices = {}
    for g in range(n_model):
        wq_g = wq[:, g * C.NQD:(g + 1) * C.NQD]
        # [nKB, HQ, 128, KB*128]: wq_r[kb, h, p, ki*128+c]
        #   = wq[(kb*KB+ki)*128 + p, h*128 + c]
        wq_r = np.ascontiguousarray(
            wq_g.reshape(C.nKB, C.KB, 128, C.HQ, 128)
            .transpose(0, 3, 2, 1, 4)
            .reshape(C.nKB, C.HQ, 128, C.KB * 128)).astype(sd)
        wk_r = np.ascontiguousarray(
            wk[:, g * C.NKD:(g + 1) * C.NKD].reshape(C.KT, 128, C.NKD)).astype(sd)
        wv_r = np.ascontiguousarray(
            wv[:, g * C.NKD:(g + 1) * C.NKD].reshape(C.KT, 128, C.NKD)).astype(sd)
        wo_gs = np.ascontiguousarray(wo[g * C.NQD:(g + 1) * C.NQD, :]).astype(sd)
        bqh = np.ascontiguousarray(bq[g * C.NQD:(g + 1) * C.NQD].reshape(C.HQ, 128).T)
        bkh = np.ascontiguousarray(bk[g * C.NKD:(g + 1) * C.NKD].reshape(C.HKV, 128).T)
        gslices[g] = (wq_r, wk_r, wv_r, wo_gs, bqh, bkh)

    in_maps = []
    for core in range(B * n_model):
        b, g = divmod(core, n_model)
        wq_r, wk_r, wv_r, wo_gs, bqh, bkh = gslices[g]
        in_maps.append({
            "xqT": xT[b][0], "xkT": xT[b][1], "xvT": xT[b][2],
            "wq_r": wq_r, "wk_r": wk_r, "wv_r": wv_r, "wo_g": wo_gs,
            "cosT": cosT.astype(sd), "sinrT": sinrT.astype(sd),
            "trimask": trimask.astype(sd),
            "bqh": bqh, "bkh": bkh,
            "ones_d": np.ones((128, 1), dtype=sd),
            "ident_d": np.eye(128, dtype=np.float32).astype(sd),
            # rotate-half permutation: out[i] = x[(i+64)%128]
            "rot_d": np.ascontiguousarray(
                np.roll(np.eye(128, dtype=np.float32), 64, axis=0)).astype(sd),
        })
    return in_maps


def assemble_output(C: Cfg, results, B, n_model, bv, wo):
    D = C.D
    out = np.zeros((B, C.T, D), dtype=np.float32)
    for core in range(B * n_model):
        b, g = divmod(core, n_model)
        out[b] += results[core]["out"]
        out[b] += results[core]["out2"]
    # bias_v enters linearly: rows of normalized attn weights sum to 1, so
    # O = P@V + 1*bv_exp^T exactly; fold the rank-1 term through wo on host.
    bv = np.asarray(bv, dtype=np.float32)
    wo = np.asarray(wo, dtype=np.float32)
    if np.any(bv):
        corr = np.zeros((D,), dtype=np.float32)
        for g in range(n_model):
            bv_g = bv[g * C.NKD:(g + 1) * C.NKD]
            bvexp = np.empty((C.NQD,), dtype=np.float32)
            for h in range(C.HQ):
                kvl = h // C.GRP
                bvexp[h * 128:(h + 1) * 128] = bv_g[kvl * 128:(kvl + 1) * 128]
            corr += bvexp @ wo[g * C.NQD:(g + 1) * C.NQD, :]
        out += corr[None, None, :]
    return out


_PROG_CACHE = {}


def get_program(C: Cfg = FULL):
    key = C
    if key not in _PROG_CACHE:
        _PROG_CACHE[key] = build_program(C)
    return _PROG_CACHE[key]


def kernel(query, key, value, mask, wq, bq, wk, bk, wv, bv, wo):
    C = FULL
    B = query.shape[0]
    n_model = (wq.shape[1] // HD) // C.HQ
    n_cores = B * n_model
    nc = get_program(C)
    in_maps = build_in_maps(C, query, key, value, mask, wq, bq, wk, bk, wv, bv, wo,
                            n_model)
    res = bass_utils.run_bass_kernel_spmd(nc, in_maps, core_ids=list(range(n_cores)))
    return assemble_output(C, res.results, B, n_model, bv, wo)
